# revision 14
# baseline (speedup 1.0000x reference)
"""Bass/Trainium2 kernel for the EquivariantPsuedoDecoder GNN (PaiNN-style).

Self-contained: host-side index preprocessing + bass program builder + SPMD
runner over 8 NeuronCores.

Sharding: core c owns nodes [NOWN*c, NOWN*(c+1)) (N padded to NPAD); every
edge is assigned to the core owning its src node, grouped by 128-node source
block so segment-sums become one-hot matmuls accumulated in PSUM. Node state
(H, Sbar feature-major; V, Vbar node-major) is exchanged with per-layer
AllGathers. phi[dst] / V[dst] / Vbar[dst] are fetched with dma_gather.
"""
import math
import numpy as np

import concourse.bass as bass
import concourse.bacc as bacc
import concourse.tile as tile
import concourse.mybir as mybir
from concourse import bass_utils

F32 = mybir.dt.float32
I16 = mybir.dt.int16
I32 = mybir.dt.int32
AF = mybir.ActivationFunctionType
OP = mybir.AluOpType

# ---- problem constants (hardcoded per spec) ----
N_NODES = 10000
F = 128
NRBF = 20
CUTOFF = 5.0
L = 3
EPS = 1e-15
NCORES = 8
NPAD = 10240            # N padded to 128*NCORES multiple
NOWN = NPAD // NCORES   # nodes owned per core (1280)
NBLK = NOWN // 128      # 128-node source blocks per core (10)
CH_T = 6                # tiles (of 128 edges) per gather chunk
SIM_SAFE_SILU = False   # CoreSim lacks Silu; emulate with Sigmoid*x when set
DEBUG_DUMPS = False     # add per-stage debug outputs


def _silu(nc, pool, out_ap, in_ap, bias_ap, tag):
    if not SIM_SAFE_SILU:
        nc.scalar.activation(out_ap, in_ap, AF.Silu, bias=bias_ap)
        return
    shape = list(out_ap.shape)
    sig = pool.tile(shape, F32, tag=tag + "_sg", name=tag + "_sg")
    nc.scalar.activation(sig[:], in_ap, AF.Sigmoid, bias=bias_ap)
    tmp = pool.tile(shape, F32, tag=tag + "_tm", name=tag + "_tm")
    nc.vector.tensor_scalar(tmp[:], in_ap, bias_ap, None, OP.add)
    nc.vector.tensor_tensor(out_ap, sig[:], tmp[:], OP.mult)


def _newton_sqrt(nc, pool, y_ap, x_ap, shape, tag, steps=2):
    """Refine y ~= sqrt(x) in place: y <- 0.5*(y + x/y)."""
    for it in range(steps):
        r = pool.tile(shape, F32, tag=f"{tag}_r", name=f"{tag}_r")
        nc.vector.reciprocal(r[:], y_ap)
        t = pool.tile(shape, F32, tag=f"{tag}_t", name=f"{tag}_t")
        nc.vector.tensor_tensor(t[:], x_ap, r[:], OP.mult)
        nc.vector.tensor_tensor(t[:], t[:], y_ap, OP.add)
        nc.vector.tensor_scalar(y_ap, t[:], 0.5, None, OP.mult)


def configure(n_nodes, npad):
    """Override problem size (for scaled-down simulator testing)."""
    global N_NODES, NPAD, NOWN, NBLK
    N_NODES = n_nodes
    NPAD = npad
    NOWN = NPAD // NCORES
    NBLK = max(1, NOWN // 128)


# ======================================================================
# host-side preprocessing
# ======================================================================

def _wrap16(x: np.ndarray) -> np.ndarray:
    """(E,) -> int16 [128, E/16]; idx i at [i%16, i//16], replicated x8."""
    e = x.shape[0]
    w = x.astype(np.int16).reshape(e // 16, 16).T
    return np.ascontiguousarray(np.tile(w, (8, 1)))


def preprocess(cg_xyz, nbr):
    src = np.asarray(nbr[:, 0], dtype=np.int64)
    dst = np.asarray(nbr[:, 1], dtype=np.int64)
    xyz = np.asarray(cg_xyz, dtype=np.float32)

    gblk = src // 128                        # global 128-node source block
    counts = np.bincount(gblk, minlength=NPAD // 128)
    t_b = max(1, int(math.ceil(counts.max() / 128)))
    e_c = NBLK * t_b * 128                  # edges per core (padded)
    t_tot = e_c // 128

    order = np.argsort(gblk, kind="stable")
    # per-block start offsets in the sorted edge list
    blk_start = np.zeros(NPAD // 128 + 1, dtype=np.int64)
    np.cumsum(counts, out=blk_start[1:])

    per_core = []
    for c in range(NCORES):
        dst_e = np.zeros(e_c, dtype=np.int64)
        srcl_e = np.full(e_c, -1.0, dtype=np.float32)
        xs = np.zeros((e_c, 3), dtype=np.float32)
        xd = np.zeros((e_c, 3), dtype=np.float32)
        xd[:, 0] = 2.0 * CUTOFF              # pad edges: dist=10 -> env=0
        for bl in range(NBLK):
            gb = c * NBLK + bl
            sel = order[blk_start[gb]:blk_start[gb + 1]]
            n = sel.shape[0]
            base = bl * t_b * 128
            assert n <= t_b * 128
            dst_e[base:base + n] = dst[sel]
            srcl_e[base:base + n] = (src[sel] - gb * 128).astype(np.float32)
            xs[base:base + n] = xyz[src[sel]]
            xd[base:base + n] = xyz[dst[sel]]
        per_core.append(dict(
            dstw=_wrap16(dst_e),
            srclw=np.ascontiguousarray(srcl_e.reshape(t_tot, 128).T),
            xyzs=np.ascontiguousarray(xs.reshape(t_tot, 128, 3).transpose(1, 0, 2)),
            xyzd=np.ascontiguousarray(xd.reshape(t_tot, 128, 3).transpose(1, 0, 2)),
        ))
    return per_core, t_b, e_c


def pack_weights(inp):
    """Host-side packing of the dense-layer weights (shared by all cores)."""
    f32 = lambda x: np.ascontiguousarray(np.asarray(x, dtype=np.float32))
    wt = np.concatenate([np.asarray(inp["dist_W"], np.float32),
                         np.asarray(inp["dist_b"], np.float32)[:, None, :]], axis=1)
    return dict(
        msgW1=f32(inp["msg_W1"]),                       # (L,256,256)
        msgb1=f32(np.asarray(inp["msg_b1"])[..., None]),  # (L,256,1)
        msgW2=f32(inp["msg_W2"]),                       # (L,256,640)
        msgb2=f32(np.asarray(inp["msg_b2"])[:, None, :]),  # (L,1,640)
        Wt=f32(wt),                                     # (L,21,640)
        updU=f32(inp["upd_U"]),                         # (L,128,128)
        updV=f32(inp["upd_V"]),
        updW1=f32(inp["upd_W1"]),                       # (L,256,128)
        updb1=f32(np.asarray(inp["upd_b1"])[..., None]),  # (L,128,1)
        updW2=f32(inp["upd_W2"]),                       # (L,128,384)
        updb2=f32(np.asarray(inp["upd_b2"])[..., None]),  # (L,384,1)
    )


def prep_h0(H):
    """H (N,F) -> feature-major [128, NPAD] + per-core own slices."""
    h = np.zeros((F, NPAD), dtype=np.float32)
    h[:, :N_NODES] = np.asarray(H, dtype=np.float32).T
    return np.ascontiguousarray(h)


# ======================================================================
# bass program
# ======================================================================

def build_program(t_b, e_c):
    t_tot = e_c // 128
    f5 = 5 * F    # 640
    f3 = 3 * F    # 384

    nc = bacc.Bacc("TRN2", target_bir_lowering=False, debug=False,
                   enable_asserts=True, num_devices=NCORES)

    din = lambda name, shape, dt=F32: nc.dram_tensor(name, shape, dt, kind="ExternalInput").ap()
    dout = lambda name, shape, dt=F32: nc.dram_tensor(name, shape, dt, kind="ExternalOutput").ap()

    # ---- inputs
    H0full = din("H0full", [F, NPAD])
    H0own = din("H0own", [F, NOWN])
    xyzs = din("xyzs", [128, t_tot, 3])
    xyzd = din("xyzd", [128, t_tot, 3])
    dstw = din("dstw", [128, e_c // 16], I16)
    srclw = din("srclw", [128, t_tot])
    w_in = {k: din(k, list(s)) for k, s in dict(
        msgW1=(L, 2 * F, 2 * F), msgb1=(L, 2 * F, 1), msgW2=(L, 2 * F, f5),
        msgb2=(L, 1, f5), Wt=(L, NRBF + 1, f5), updU=(L, F, F), updV=(L, F, F),
        updW1=(L, 2 * F, F), updb1=(L, F, 1), updW2=(L, F, f3), updb2=(L, f3, 1),
    ).items()}

    # ---- outputs
    Hout = dout("Hout", [F, NOWN])
    Vout = dout("Vout", [NOWN, f3])
    dbg = {}
    if DEBUG_DUMPS:
        dbg["B"] = dout("dbgB", [NRBF + 1, e_c])
        dbg["unit"] = dout("dbgU", [128, t_tot, 3])
        dbg["phi0"] = dout("dbgphi0", [NPAD, f5])
        for dl in range(2):
            dbg[f"H{dl}"] = dout(f"dbgH{dl}", [F, NOWN])
            dbg[f"S{dl}"] = dout(f"dbgS{dl}", [F, NOWN])
            dbg[f"V{dl}"] = dout(f"dbgV{dl}", [NOWN, f3])
        dbg["VB0"] = dout("dbgVB0", [NOWN, f3])

    with tile.TileContext(nc) as tc:
        with (
            tc.tile_pool(name="persist", bufs=1) as pp,
            tc.tile_pool(name="dram", bufs=1, space="DRAM") as dp,
        ):
            # ---------- DRAM internals ----------
            phi_d = [dp.tile([NPAD, f5], F32, name=f"phi_d{i}") for i in range(2)]
            B_d = dp.tile([NRBF + 1, e_c], F32)
            Hown_d = [dp.tile([F, NOWN], F32, name=f"Hown{i}") for i in range(2)]
            Sown_d = [dp.tile([F, NOWN], F32, name=f"Sown{i}") for i in range(2)]
            Vown_d = [dp.tile([NOWN, f3], F32, name=f"Vown{i}") for i in range(2)]
            VBown_d = dp.tile([NOWN, f3], F32)
            Hfull = {l: dp.tile([NCORES, F, NOWN], F32, addr_space="Shared", name=f"Hfull{l}") for l in (1, 2)}
            Sfull = {l: dp.tile([NCORES, F, NOWN], F32, addr_space="Shared", name=f"Sfull{l}") for l in (1, 2)}
            Vfull = {l: dp.tile([NPAD, f3], F32, addr_space="Shared", name=f"Vfull{l}") for l in (1, 2)}
            VBfull = dp.tile([NPAD, f3], F32, addr_space="Shared")

            # ---------- persistent SBUF ----------
            dst_sb = pp.tile([128, e_c // 16], I16)
            nc.sync.dma_start(dst_sb[:], dstw)
            srcl_sb = pp.tile([128, t_tot], F32)
            nc.sync.dma_start(srcl_sb[:], srclw)
            unit_sb = pp.tile([128, t_tot, 3], F32)

            # constants
            iota_i = pp.tile([128, 128], I32)
            nc.gpsimd.iota(iota_i[:], pattern=[[1, 128]], base=0, channel_multiplier=0)
            iotaf = pp.tile([128, 128], F32)
            nc.vector.tensor_copy(iotaf[:], iota_i[:])
            pid_i = pp.tile([128, 1], I32)
            nc.gpsimd.iota(pid_i[:], pattern=[[0, 1]], base=0, channel_multiplier=1)
            pidf = pp.tile([128, 1], F32)
            nc.vector.tensor_copy(pidf[:], pid_i[:])
            ident = pp.tile([128, 128], F32)
            nc.vector.tensor_tensor(ident[:], pidf[:].broadcast_to([128, 128]),
                                    iotaf[:], OP.is_equal)
            ones1 = pp.tile([1, 128], F32)
            nc.vector.memset(ones1[:], 1.0)
            halfpi = pp.tile([128, 1], F32)
            nc.vector.memset(halfpi[:], float(np.pi / 2))
            eps_b = pp.tile([128, 1], F32)
            nc.vector.memset(eps_b[:], float(EPS))

            # deltas (per layer, reused)
            dH_sb = pp.tile([128, NBLK, F], F32)
            dS_sb = pp.tile([128, NBLK, F], F32)
            dV_sb = pp.tile([128, NBLK, F, 3], F32)
            dVB_sb = pp.tile([128, NBLK, F, 3], F32)

            # ---------- geometry precompute (once) ----------
            with (
                tc.tile_pool(name="geo_sb", bufs=1) as gs,
                tc.tile_pool(name="geo_ps", bufs=2, space="PSUM") as gp,
            ):
                xs_sb = gs.tile([128, t_tot, 3], F32)
                nc.sync.dma_start(xs_sb[:], xyzs)
                xd_sb = gs.tile([128, t_tot, 3], F32)
                nc.sync.dma_start(xd_sb[:], xyzd)
                r = gs.tile([128, t_tot, 3], F32)
                nc.vector.tensor_tensor(r[:], xd_sb[:], xs_sb[:], OP.subtract)
                r2 = gs.tile([128, t_tot, 3], F32)
                nc.vector.tensor_tensor(r2[:], r[:], r[:], OP.mult)
                d2 = gs.tile([128, t_tot], F32)
                nc.vector.tensor_reduce(d2[:], r2[:], mybir.AxisListType.X, OP.add)
                dist = gs.tile([128, t_tot], F32)
                nc.scalar.activation(dist[:], d2[:], AF.Sqrt)
                _newton_sqrt(nc, gs, dist[:], d2[:], [128, t_tot], "dnw", steps=1)
                rinv = gs.tile([128, t_tot], F32)
                nc.vector.reciprocal(rinv[:], dist[:])
                nc.vector.tensor_tensor(
                    unit_sb[:], r[:],
                    rinv[:].unsqueeze(2).broadcast_to([128, t_tot, 3]), OP.mult)
                dcl = gs.tile([128, t_tot], F32)
                nc.vector.tensor_scalar(dcl[:], dist[:], float(CUTOFF), None, OP.min)
                s_cur = gs.tile([128, t_tot], F32)
                nc.scalar.activation(s_cur[:], dcl[:], AF.Sin, scale=float(np.pi / CUTOFF))
                cosx = gs.tile([128, t_tot], F32)
                nc.scalar.activation(cosx[:], dcl[:], AF.Sin, bias=halfpi[:],
                                     scale=float(-np.pi / CUTOFF))
                env = gs.tile([128, t_tot], F32)
                nc.vector.tensor_scalar(env[:], cosx[:], 0.5, 0.5, OP.mult, OP.add)
                mask = gs.tile([128, t_tot], F32)
                nc.vector.tensor_scalar(mask[:], dist[:], float(CUTOFF), None, OP.is_lt)
                nc.vector.tensor_tensor(env[:], env[:], mask[:], OP.mult)
                wgt = gs.tile([128, t_tot], F32)
                nc.vector.tensor_tensor(wgt[:], env[:], rinv[:], OP.mult)

                # Chebyshev: sin(k x) planes, weighted rows into B_em [128, t, 21]
                B_em = gs.tile([128, t_tot, NRBF + 1], F32)
                s_pl = gs.tile([128, NRBF, t_tot], F32)
                nc.vector.tensor_copy(s_pl[:, 0, :], s_cur[:])
                nc.vector.tensor_tensor(B_em[:, :, 0], s_cur[:], wgt[:], OP.mult)
                for k in range(1, NRBF):
                    nc.vector.scalar_tensor_tensor(
                        s_pl[:, k, :], cosx[:], 2.0, s_pl[:, k - 1, :],
                        OP.mult, OP.mult)
                    if k >= 2:
                        nc.vector.tensor_tensor(
                            s_pl[:, k, :], s_pl[:, k, :], s_pl[:, k - 2, :],
                            OP.subtract)
                    nc.vector.tensor_tensor(
                        B_em[:, :, k], s_pl[:, k, :], wgt[:], OP.mult)
                nc.vector.tensor_copy(B_em[:, :, NRBF], env[:])

                # transpose per tile -> B_d [21, e_c]
                for t in range(t_tot):
                    tp = gp.tile([NRBF + 1, 128], F32, tag="btp")
                    nc.tensor.transpose(tp[:], B_em[:, t, :], ident[:])
                    bs = gs.tile([NRBF + 1, 128], F32, tag="bsb")
                    nc.vector.tensor_copy(bs[:], tp[:])
                    nc.sync.dma_start(B_d[:, t * 128:(t + 1) * 128], bs[:])
                    if DEBUG_DUMPS:
                        nc.sync.dma_start(dbg["B"][:, t * 128:(t + 1) * 128], bs[:])
                if DEBUG_DUMPS:
                    nc.sync.dma_start(dbg["unit"], unit_sb[:])

            # ---------- layers ----------
            for l in range(L):
                _layer(nc, tc, l, t_b, t_tot, e_c,
                       w_in, H0full, H0own, dst_sb, srcl_sb, unit_sb,
                       iotaf, ident, ones1, halfpi, eps_b,
                       dH_sb, dS_sb, dV_sb, dVB_sb,
                       phi_d, B_d, Hown_d, Sown_d, Vown_d, VBown_d,
                       Hfull, Sfull, Vfull, VBfull, Hout, Vout, dbg)

    nc.compile()
    return nc


def _layer(nc, tc, l, t_b, t_tot, e_c,
           w_in, H0full, H0own, dst_sb, srcl_sb, unit_sb,
           iotaf, ident, ones1, halfpi, eps_b,
           dH_sb, dS_sb, dV_sb, dVB_sb,
           phi_d, B_d, Hown_d, Sown_d, Vown_d, VBown_d,
           Hfull, Sfull, Vfull, VBfull, Hout, Vout, dbg):
    f5, f3 = 5 * F, 3 * F
    ntile = NPAD // 128
    phi = phi_d[l % 2]

    # ================= phase A: phi MLP over all nodes =================
    with (
        tc.tile_pool(name=f"A_sb{l}", bufs=2) as sa,
        tc.tile_pool(name=f"A_w{l}", bufs=1) as wa,
        tc.tile_pool(name=f"A_ps{l}", bufs=2, space="PSUM") as pa,
    ):
        w1 = {}
        for ks in range(2):
            for ms in range(2):
                w1[ks, ms] = wa.tile([128, 128], F32, tag=f"w1_{ks}{ms}", name=f"w1_{ks}{ms}")
                nc.sync.dma_start(
                    w1[ks, ms][:],
                    w_in["msgW1"][l, ks * 128:(ks + 1) * 128, ms * 128:(ms + 1) * 128])
        b1 = {}
        for ms in range(2):
            b1[ms] = wa.tile([128, 1], F32, tag=f"b1_{ms}", name=f"b1_{ms}")
            nc.sync.dma_start(b1[ms][:], w_in["msgb1"][l, ms * 128:(ms + 1) * 128, :])
        w2 = {}
        for ks in range(2):
            w2[ks] = wa.tile([128, f5], F32, tag=f"w2_{ks}", name=f"w2_{ks}")
            nc.sync.dma_start(w2[ks][:], w_in["msgW2"][l, ks * 128:(ks + 1) * 128, :])
        b2row = wa.tile([1, f5], F32, tag="b2row")
        nc.sync.dma_start(b2row[:], w_in["msgb2"][l, :, :])

        for nt in range(ntile):
            if l == 0:
                rhs_h = sa.tile([128, 128], F32, tag="rhs_h")
                nc.sync.dma_start(rhs_h[:], H0full[:, nt * 128:(nt + 1) * 128])
                rhs_s = None
            else:
                r_, lt_ = nt // NBLK, nt % NBLK
                rhs_h = sa.tile([128, 128], F32, tag="rhs_h")
                nc.sync.dma_start(rhs_h[:], Hfull[l][r_, :, lt_ * 128:(lt_ + 1) * 128])
                rhs_s = sa.tile([128, 128], F32, tag="rhs_s")
                nc.sync.dma_start(rhs_s[:], Sfull[l][r_, :, lt_ * 128:(lt_ + 1) * 128])

            acts = []
            for ms in range(2):
                ps1 = pa.tile([128, 128], F32, tag=f"ps1_{ms}")
                nc.tensor.matmul(ps1[:], w1[0, ms][:], rhs_h[:],
                                 start=True, stop=(l == 0))
                if l > 0:
                    nc.tensor.matmul(ps1[:], w1[1, ms][:], rhs_s[:],
                                     start=False, stop=True)
                a_ = sa.tile([128, 128], F32, tag=f"act1_{ms}", name=f"act1_{ms}")
                _silu(nc, sa, a_[:], ps1[:], b1[ms][:], f"slA{ms}")
                acts.append(a_)

            php = pa.tile([128, f5], F32, tag="php")
            for n0, n1 in ((0, 512), (512, f5)):
                nc.tensor.matmul(php[:, n0:n1], acts[0][:], w2[0][:, n0:n1],
                                 start=True, stop=False)
                nc.tensor.matmul(php[:, n0:n1], acts[1][:], w2[1][:, n0:n1],
                                 start=False, stop=False)
                nc.tensor.matmul(php[:, n0:n1], ones1[:], b2row[:, n0:n1],
                                 start=False, stop=True)
            phs = sa.tile([128, f5], F32, tag="phs")
            nc.vector.tensor_copy(phs[:], php[:])
            nc.sync.dma_start(phi[nt * 128:(nt + 1) * 128, :], phs[:])
            if DEBUG_DUMPS and l == 0:
                nc.sync.dma_start(dbg["phi0"][nt * 128:(nt + 1) * 128, :], phs[:])

    # ================= phase B: edge messages + segment sums ============
    n_t = NBLK * t_b
    # psum channel layout: [ds(0:128) | V(128:512)] bank0, [dsbar | VB] bank1
    ch_l = 768 if l == 0 else (1024 if l == 1 else 512)
    with (
        tc.tile_pool(name=f"B_sb{l}", bufs=2) as sb,
        tc.tile_pool(name=f"B_w{l}", bufs=1) as wb,
        tc.tile_pool(name=f"B_ps{l}", bufs=2, space="PSUM") as pb,
        tc.tile_pool(name=f"B_blk{l}", bufs=2, space="PSUM") as pblk,
    ):
        wt = wb.tile([NRBF + 1, f5], F32, tag="wt")
        nc.sync.dma_start(wt[:], w_in["Wt"][l])

        blk = None
        for c0 in range(0, n_t, CH_T):
            k = min(CH_T, n_t - c0)
            ne = 128 * k
            gphi = sb.tile([128, CH_T, f5], F32, tag="gphi")
            nc.gpsimd.dma_gather(
                out_ap=gphi[:, 0:k, :], in_ap=phi[:],
                idxs_ap=dst_sb[:, c0 * 8:(c0 + k) * 8],
                num_idxs=ne, num_idxs_reg=ne, elem_size=f5)
            if l >= 1:
                gV = sb.tile([128, CH_T, f3], F32, tag="gV")
                nc.gpsimd.dma_gather(
                    out_ap=gV[:, 0:k, :], in_ap=Vfull[l][:],
                    idxs_ap=dst_sb[:, c0 * 8:(c0 + k) * 8],
                    num_idxs=ne, num_idxs_reg=ne, elem_size=f3)
            if l == 1:
                gVB = sb.tile([128, CH_T, f3], F32, tag="gVB")
                nc.gpsimd.dma_gather(
                    out_ap=gVB[:, 0:k, :], in_ap=VBfull[:],
                    idxs_ap=dst_sb[:, c0 * 8:(c0 + k) * 8],
                    num_idxs=ne, num_idxs_reg=ne, elem_size=f3)
            bx = sb.tile([NRBF + 1, CH_T * 128], F32, tag="bx")
            nc.sync.dma_start(bx[:, 0:ne], B_d[:, c0 * 128:(c0 + k) * 128])

            for i in range(k):
                t = c0 + i
                b = t // t_b
                fi = (t % t_b == 0)
                la = (t % t_b == t_b - 1)

                oh = sb.tile([128, 128], F32, tag="oh")
                nc.vector.tensor_tensor(
                    oh[:], srcl_sb[:, t:t + 1].broadcast_to([128, 128]),
                    iotaf[:], OP.is_equal)

                ws = pb.tile([128, f5], F32, tag="ws")
                nc.tensor.matmul(ws[:, 0:512], bx[:, i * 128:(i + 1) * 128],
                                 wt[:, 0:512], start=True, stop=True)
                nc.tensor.matmul(ws[:, 512:f5], bx[:, i * 128:(i + 1) * 128],
                                 wt[:, 512:f5], start=True, stop=True)

                inv = sb.tile([128, f5], F32, tag="inv")
                nc.vector.tensor_tensor(inv[:], gphi[:, i, :], ws[:], OP.mult)

                ua = sb.tile([128, F, 3], F32, tag="ua")
                nc.vector.tensor_tensor(
                    ua[:], inv[:, 256:384].unsqueeze(2).broadcast_to([128, F, 3]),
                    unit_sb[:, t, :].unsqueeze(1).broadcast_to([128, F, 3]),
                    OP.mult)
                if l >= 1:
                    gvv = sb.tile([128, F, 3], F32, tag="gvv")
                    nc.vector.tensor_tensor(
                        gvv[:], gV[:, i, :].rearrange("p (f d) -> p f d", d=3),
                        inv[:, 0:128].unsqueeze(2).broadcast_to([128, F, 3]),
                        OP.mult)
                if l == 1:
                    gvb = sb.tile([128, F, 3], F32, tag="gvb")
                    nc.vector.tensor_tensor(
                        gvb[:], gVB[:, i, :].rearrange("p (f d) -> p f d", d=3),
                        inv[:, 512:640].unsqueeze(2).broadcast_to([128, F, 3]),
                        OP.mult)

                if fi:
                    blk = pblk.tile([128, ch_l], F32, tag="blk")
                    nc.vector.memset(blk[:], 0.0)
                mm = lambda o, r: nc.tensor.matmul(
                    o, oh[:], r, start=False, stop=False, skip_group_check=True)
                mm(blk[:, 0:128], inv[:, 128:256])
                mm(blk[:, 128:512], ua[:])
                if l >= 1:
                    mm(blk[:, 128:512], gvv[:])
                if l < 2:
                    mm(blk[:, 512:640], inv[:, 384:512])
                if l == 1:
                    mm(blk[:, 640:1024], ua[:])
                    mm(blk[:, 640:1024], gvb[:])

                if la:
                    nc.vector.tensor_copy(dH_sb[:, b, :], blk[:, 0:128])
                    nc.scalar.copy(
                        dV_sb[:, b, :, :].rearrange("p f d -> p (f d)"),
                        blk[:, 128:512])
                    if l < 2:
                        nc.vector.tensor_copy(dS_sb[:, b, :], blk[:, 512:640])
                    if l == 1:
                        nc.scalar.copy(
                            dVB_sb[:, b, :, :].rearrange("p f d -> p (f d)"),
                            blk[:, 640:1024])

    # ================= phase C: update block on own nodes ===============
    with (
        tc.tile_pool(name=f"C_sb{l}", bufs=2) as sc,
        tc.tile_pool(name=f"C_w{l}", bufs=1) as wc,
        tc.tile_pool(name=f"C_ps{l}", bufs=4, space="PSUM") as pc,
    ):
        uU = wc.tile([128, 128], F32, tag="uU")
        nc.sync.dma_start(uU[:], w_in["updU"][l])
        uV = wc.tile([128, 128], F32, tag="uV")
        nc.sync.dma_start(uV[:], w_in["updV"][l])
        uW1 = {}
        for ks in range(2):
            uW1[ks] = wc.tile([128, 128], F32, tag=f"uW1_{ks}", name=f"uW1_{ks}")
            nc.sync.dma_start(uW1[ks][:], w_in["updW1"][l, ks * 128:(ks + 1) * 128, :])
        ub1 = wc.tile([128, 1], F32, tag="ub1")
        nc.sync.dma_start(ub1[:], w_in["updb1"][l])
        uW2 = {}
        ub2 = {}
        for ms in range(3):
            uW2[ms] = wc.tile([128, 128], F32, tag=f"uW2_{ms}", name=f"uW2_{ms}")
            nc.sync.dma_start(uW2[ms][:], w_in["updW2"][l, :, ms * 128:(ms + 1) * 128])
            ub2[ms] = wc.tile([128, 1], F32, tag=f"ub2_{ms}", name=f"ub2_{ms}")
            nc.sync.dma_start(ub2[ms][:], w_in["updb2"][l, ms * 128:(ms + 1) * 128, :])

        for lt in range(NBLK):
            sl = slice(lt * 128, (lt + 1) * 128)
            # H_new = H_prior + dH^T (feature-major)
            hp = sc.tile([128, 128], F32, tag="hp")
            hsrc = H0own if l == 0 else Hown_d[(l - 1) % 2][:]
            nc.sync.dma_start(hp[:], hsrc[:, sl])
            dhT = pc.tile([128, 128], F32, tag="cpsa")
            nc.tensor.transpose(dhT[:], dH_sb[:, lt, :], ident[:])
            hnew = sc.tile([128, 128], F32, tag="hnew")
            nc.vector.tensor_tensor(hnew[:], hp[:], dhT[:], OP.add)
            # Sbar_new
            if l < 2:
                dsT = pc.tile([128, 128], F32, tag="cpsa")
                nc.tensor.transpose(dsT[:], dS_sb[:, lt, :], ident[:])
                snew = sc.tile([128, 128], F32, tag="snew")
                if l == 0:
                    nc.vector.tensor_copy(snew[:], dsT[:])
                else:
                    sp = sc.tile([128, 128], F32, tag="sp")
                    nc.sync.dma_start(sp[:], Sown_d[0][:][:, sl])
                    nc.vector.tensor_tensor(snew[:], sp[:], dsT[:], OP.add)
            # V_mid = V_prior + dV (node-major [n, f, d])
            vmid = sc.tile([128, F, 3], F32, tag="vmid")
            if l == 0:
                nc.vector.tensor_copy(vmid[:], dV_sb[:, lt, :, :])
            else:
                vp = sc.tile([128, F, 3], F32, tag="vp")
                nc.sync.dma_start(
                    vp[:].rearrange("p f d -> p (f d)"),
                    Vown_d[(l - 1) % 2][:][sl, :])
                nc.vector.tensor_tensor(vmid[:], vp[:], dV_sb[:, lt, :, :], OP.add)
            # vfm [f, n, d] via 3 PE transposes
            vfm = sc.tile([128, 128, 3], F32, tag="vfm")
            for d in range(3):
                tpd = pc.tile([128, 128], F32, tag="cpsa")
                nc.tensor.transpose(tpd[:], vmid[:, :, d], ident[:])
                nc.vector.tensor_copy(vfm[:, :, d], tpd[:])
            # u_v, v_v [g, n, d]
            uvp = pc.tile([128, 128, 3], F32, tag="cpsb")
            nc.tensor.matmul(uvp[:], uU[:], vfm[:], start=True, stop=True)
            vvp = pc.tile([128, 128, 3], F32, tag="cpsb")
            nc.tensor.matmul(vvp[:], uV[:], vfm[:], start=True, stop=True)
            uvs = sc.tile([128, 128, 3], F32, tag="uvs")
            nc.scalar.copy(uvs[:], uvp[:])
            vvs = sc.tile([128, 128, 3], F32, tag="vvs")
            nc.scalar.copy(vvs[:], vvp[:])
            # v_norm
            vv2 = sc.tile([128, 128, 3], F32, tag="vv2")
            nc.vector.tensor_tensor(vv2[:], vvs[:], vvs[:], OP.mult)
            vn = sc.tile([128, 128], F32, tag="vn")
            nc.vector.tensor_reduce(vn[:], vv2[:], mybir.AxisListType.X, OP.add)
            nc.vector.tensor_scalar(vn[:], vn[:], float(EPS), None, OP.add)
            vn2 = sc.tile([128, 128], F32, tag="vn2")
            nc.scalar.activation(vn2[:], vn[:], AF.Sqrt)
            _newton_sqrt(nc, sc, vn2[:], vn[:], [128, 128], "vnw", steps=2)
            # dot = sum_d u_v * v_v
            duv = sc.tile([128, 128, 3], F32, tag="duv")
            nc.vector.tensor_tensor(duv[:], uvs[:], vvs[:], OP.mult)
            dot = sc.tile([128, 128], F32, tag="dot")
            nc.vector.tensor_reduce(dot[:], duv[:], mybir.AxisListType.X, OP.add)
            # a-MLP
            ap1 = pc.tile([128, 128], F32, tag="cpsa")
            nc.tensor.matmul(ap1[:], uW1[0][:], hnew[:], start=True, stop=False)
            nc.tensor.matmul(ap1[:], uW1[1][:], vn2[:], start=False, stop=True)
            a1a = sc.tile([128, 128], F32, tag="a1a")
            _silu(nc, sc, a1a[:], ap1[:], ub1[:], "slC")
            am = []
            for ms in range(3):
                ap2 = pc.tile([128, 128], F32, tag="cpsa")
                nc.tensor.matmul(ap2[:], uW2[ms][:], a1a[:], start=True, stop=True)
                a_ = sc.tile([128, 128], F32, tag=f"am_{ms}")
                nc.vector.tensor_scalar(a_[:], ap2[:], ub2[ms][:], None, OP.add)
                am.append(a_)
            # dV_u = u_v * a0 ; dH_u = a1*dot + a2
            dvu = sc.tile([128, 128, 3], F32, tag="dvu")
            nc.vector.tensor_tensor(
                dvu[:], uvs[:], am[0][:].unsqueeze(2).broadcast_to([128, 128, 3]),
                OP.mult)
            dhu = sc.tile([128, 128], F32, tag="dhu")
            nc.vector.tensor_tensor(dhu[:], am[1][:], dot[:], OP.mult)
            nc.vector.tensor_tensor(dhu[:], dhu[:], am[2][:], OP.add)
            hfin = sc.tile([128, 128], F32, tag="hfin")
            nc.vector.tensor_tensor(hfin[:], hnew[:], dhu[:], OP.add)
            # V_final = V_mid + dV_u^T (node-major)
            vfin = sc.tile([128, F, 3], F32, tag="vfin")
            for d in range(3):
                tq = pc.tile([128, 128], F32, tag="cpsa")
                nc.tensor.transpose(tq[:], dvu[:, :, d], ident[:])
                nc.vector.tensor_tensor(vfin[:, :, d], vmid[:, :, d], tq[:], OP.add)
            # stores
            if l < 2:
                nc.sync.dma_start(Hown_d[l % 2][:][:, sl], hfin[:])
                nc.sync.dma_start(Sown_d[l % 2][:][:, sl], snew[:])
                nc.sync.dma_start(Vown_d[l % 2][:][sl, :],
                                  vfin[:].rearrange("p f d -> p (f d)"))
                if l == 0:
                    nc.sync.dma_start(VBown_d[:][sl, :],
                                      vmid[:].rearrange("p f d -> p (f d)"))
                if DEBUG_DUMPS:
                    nc.sync.dma_start(dbg[f"H{l}"][:, sl], hfin[:])
                    nc.sync.dma_start(dbg[f"S{l}"][:, sl], snew[:])
                    nc.sync.dma_start(dbg[f"V{l}"][sl, :],
                                      vfin[:].rearrange("p f d -> p (f d)"))
                    if l == 0:
                        nc.sync.dma_start(dbg["VB0"][sl, :],
                                          vmid[:].rearrange("p f d -> p (f d)"))
            else:
                nc.sync.dma_start(Hout[:, sl], hfin[:])
                nc.sync.dma_start(Vout[sl, :], vfin[:].rearrange("p f d -> p (f d)"))

    # ================= phase D: collectives =============================
    if l < 2:
        rg = [list(range(NCORES))]
        nc.gpsimd.collective_compute("AllGather", OP.bypass, replica_groups=rg,
                                     ins=[Hown_d[l % 2][:]], outs=[Hfull[l + 1][:]])
        nc.gpsimd.collective_compute("AllGather", OP.bypass, replica_groups=rg,
                                     ins=[Sown_d[l % 2][:]], outs=[Sfull[l + 1][:]])
        nc.gpsimd.collective_compute("AllGather", OP.bypass, replica_groups=rg,
                                     ins=[Vown_d[l % 2][:]], outs=[Vfull[l + 1][:]])
        if l == 0:
            nc.gpsimd.collective_compute("AllGather", OP.bypass, replica_groups=rg,
                                         ins=[VBown_d[:]], outs=[VBfull[:]])


# ======================================================================
# entry point
# ======================================================================

def prepare(inputs):
    """Host preprocessing + program build. Returns (nc, in_maps)."""
    per_core, t_b, e_c = preprocess(inputs["cg_xyz"], inputs["CG_nbr_list"])
    wts = pack_weights(inputs)
    h0full = prep_h0(inputs["H"])

    nc = build_program(t_b, e_c)

    in_maps = []
    for c in range(NCORES):
        m = dict(per_core[c])
        m["H0full"] = h0full
        m["H0own"] = np.ascontiguousarray(h0full[:, c * NOWN:(c + 1) * NOWN])
        m.update(wts)
        in_maps.append(m)
    return nc, in_maps


def assemble(results):
    h_all = np.zeros((NPAD, F), dtype=np.float32)
    v_all = np.zeros((NPAD, 3 * F), dtype=np.float32)
    for c in range(NCORES):
        h_all[c * NOWN:(c + 1) * NOWN] = results[c]["Hout"].T
        v_all[c * NOWN:(c + 1) * NOWN] = results[c]["Vout"]
    H_out = h_all[:N_NODES]
    V_out = v_all[:N_NODES].reshape(N_NODES, F, 3)
    return H_out, V_out


def _execute(inputs, trace=False, **run_kwargs):
    nc, in_maps = prepare(inputs)
    res = bass_utils.run_bass_kernel_spmd(
        nc, in_maps, core_ids=list(range(NCORES)), trace=trace, **run_kwargs)
    H_out, V_out = assemble(res.results)
    return H_out, V_out, res


def kernel(**inputs):
    H_out, V_out, _ = _execute(inputs)
    return H_out, V_out
